# revision 3
# baseline (speedup 1.0000x reference)
"""Grouped GEMM (MoE expert-parallel) on 8 TRN2 NeuronCores.

Strategy: expert-parallel — core e computes Y_e = X_e @ W_e^T for its
expert's contiguous token group.  Per core: [2048, 1024] @ [1024, 2048].

v4 design — hybrid fp8-DoubleRow / bf16 K-split:
  * Per (m, n) PSUM accumulation group: one DoubleRow fp8e4 matmul
    covers k=0..255 at 2x PE rate (measured 1.9x), then 6 bf16 matmuls
    cover k=256..1023 at 1x.  ~12% fewer PE cycles than all-bf16.
  * Both paths are pre-scaled on host by powers of two (x*32, w*1024)
    so a single fp32 PSUM group accumulates a uniform 32768x scale;
    the PSUM->SBUF copy divides it out (exact).  fp8 covers 1/4 of K,
    so quantization error is halved vs all-fp8: measured rel err
    1.603e-2 on hardware (gate 2e-2; all-fp8 would be 3.2e-2).
  * Host-pre-tiled DMA layouts: every DMA line is per-partition
    contiguous, inputs+outputs are 16-or-8-bit — DMA is fully hidden
    behind the PE (all DMA/copy/order micro-variants measured identical,
    and the full kernel matches the matmul-only floor).
  * PSUM->SBUF copies alternate scalar/vector engines.
"""

import numpy as np
import ml_dtypes

import concourse.mybir as mybir
import concourse.tile as tile
from concourse import bacc

NUM_CORES = 8
IN_F = 1024            # K (contraction)
OUT_F = 2048           # N (out features per expert)
CAP = 2048             # token capacity per core (= expected group size)
P = 128
KT = IN_F // P         # 8 k-subtiles
MT = CAP // P          # 16 m-tiles of tokens
NFREE = 512            # moving-operand free dim (one fp32 PSUM bank)
NT = OUT_F // NFREE    # 4 n-tiles
K8T = 2                # fp8 k-subtiles (one DoubleRow matmul, k=0..255)
KBT = KT - K8T         # bf16 k-subtiles (k=256..1023)
SX, SW = 32.0, 1024.0  # power-of-two pre-scales (exact in fp32/bf16)
INV_SCALE = 1.0 / (SX * SW)

DT8 = mybir.dt.float8e4
DTB = mybir.dt.bfloat16


def _build(repeat: int = 1):
    """Per-core Bass program: y[CAP, OUT_F] (bf16) = X_e @ W_e^T / 32768.

    x8: [MT, P, 2, P]      x8[m,p,j,m'] = e4m3(x[m*128+m', j*128+p]*32)
    xb: [MT, P, 6, P]      xb[m,p,j,m'] = bf16(x[m*128+m', (j+2)*128+p]*32)
    w8: [NT, P, 2, NFREE]  w8[n,p,j,f]  = e4m3(w[n*512+f, j*128+p]*1024)
    wb: [NT, P, 6, NFREE]  wb[n,p,j,f]  = bf16(w[n*512+f, (j+2)*128+p]*1024)
    """
    nc = bacc.Bacc(None, target_bir_lowering=False, debug=False)
    x8 = nc.dram_tensor("x8", [MT, P, K8T, P], DT8, kind="ExternalInput")
    xb = nc.dram_tensor("xb", [MT, P, KBT, P], DTB, kind="ExternalInput")
    w8 = nc.dram_tensor("w8", [NT, P, K8T, NFREE], DT8, kind="ExternalInput")
    wb = nc.dram_tensor("wb", [NT, P, KBT, NFREE], DTB, kind="ExternalInput")
    y = nc.dram_tensor("y", [CAP, OUT_F], DTB, kind="ExternalOutput")
    yr = y.rearrange("(mt p) n -> p mt n", p=P)

    with tile.TileContext(nc) as tc:
        with (
            tc.tile_pool(name="x8_pool", bufs=3) as x8_pool,
            tc.tile_pool(name="xb_pool", bufs=3) as xb_pool,
            tc.tile_pool(name="w_pool", bufs=2 * NT) as w_pool,
            tc.tile_pool(name="out_pool", bufs=3) as out_pool,
            tc.tile_pool(name="psum", bufs=8, space="PSUM") as psum_pool,
        ):
            for _ in range(repeat):
                wt_tiles: dict[int, tuple] = {}
                for m in range(MT):
                    x8_t = x8_pool.tile([P, K8T, P], DT8, tag="x8")
                    nc.sync.dma_start(x8_t[:], x8[m])
                    xb_t = xb_pool.tile([P, KBT, P], DTB, tag="xb")
                    nc.sync.dma_start(xb_t[:], xb[m])
                    y_sb = out_pool.tile([P, OUT_F], DTB, tag="y")
                    psums = []
                    for n in range(NT):
                        if n not in wt_tiles:
                            w8_t = w_pool.tile([P, K8T, NFREE], DT8,
                                               tag="w", name=f"w8_{n}")
                            nc.sync.dma_start(w8_t[:], w8[n])
                            wb_t = w_pool.tile([P, KBT, NFREE], DTB,
                                               tag="w", name=f"wb_{n}")
                            nc.sync.dma_start(wb_t[:], wb[n])
                            wt_tiles[n] = (w8_t, wb_t)
                        psums.append(psum_pool.tile(
                            [P, NFREE], mybir.dt.float32,
                            name=f"psum_{m}_{n}", tag="psum",
                        ))
                    # n-outer: each PSUM group's 7 matmuls back-to-back on
                    # one bank (1 DoubleRow fp8 + 6 bf16, uniform scale).
                    for n in range(NT):
                        w8_t, wb_t = wt_tiles[n]
                        nc.tensor.matmul(
                            psums[n],
                            lhsT=x8_t[:, 0:K8T, :],
                            rhs=w8_t[:, 0:K8T, :],
                            start=True, stop=False,
                            perf_mode=mybir.MatmulPerfMode.DoubleRow,
                        )
                        for o in range(KBT):
                            nc.tensor.matmul(
                                psums[n],
                                lhsT=xb_t[:, o, :],
                                rhs=wb_t[:, o, :],
                                start=False, stop=(o == KBT - 1),
                            )
                    for n in range(NT):
                        dst = y_sb[:, n * NFREE:(n + 1) * NFREE]
                        if n % 2 == 0:
                            nc.scalar.activation(
                                dst, psums[n][:],
                                mybir.ActivationFunctionType.Copy,
                                scale=INV_SCALE,
                            )
                        else:
                            nc.vector.tensor_scalar_mul(
                                dst, psums[n][:], INV_SCALE)
                    nc.sync.dma_start(yr[:, m, :], y_sb[:])
    nc.compile()
    return nc


_NC_CACHE: dict = {}


def _get_nc(repeat: int = 1):
    if repeat not in _NC_CACHE:
        _NC_CACHE[repeat] = _build(repeat)
    return _NC_CACHE[repeat]


def prep_x(xe32: np.ndarray):
    """[CAP, IN_F] fp32 -> (x8 [MT,P,2,P] e4m3, xb [MT,P,6,P] bf16)."""
    t = (xe32 * SX).reshape(MT, P, KT, P).transpose(0, 3, 2, 1)
    x8 = np.clip(t[:, :, :K8T], -240, 240).astype(ml_dtypes.float8_e4m3)
    xb = t[:, :, K8T:].astype(ml_dtypes.bfloat16)
    return np.ascontiguousarray(x8), np.ascontiguousarray(xb)


def prep_w(we32: np.ndarray):
    """[OUT_F, IN_F] fp32 -> (w8 [NT,P,2,NFREE] e4m3, wb [NT,P,6,NFREE] bf16)."""
    t = (we32 * SW).reshape(NT, NFREE, KT, P).transpose(0, 3, 2, 1)
    w8 = np.clip(t[:, :, :K8T], -240, 240).astype(ml_dtypes.float8_e4m3)
    wb = t[:, :, K8T:].astype(ml_dtypes.bfloat16)
    return np.ascontiguousarray(w8), np.ascontiguousarray(wb)


_RUNNER_CACHE: dict = {}


def _get_runner():
    """Jit the 8-core SPMD executable once; reuse across kernel() calls."""
    if "run" in _RUNNER_CACHE:
        return _RUNNER_CACHE["run"]

    import jax
    from jax.sharding import Mesh, PartitionSpec
    from jax.experimental.shard_map import shard_map
    from concourse import bass2jax
    from concourse.bass2jax import _bass_exec_p, install_neuronx_cc_hook

    nc = _get_nc(1)
    install_neuronx_cc_hook()
    assert nc.dbg_addr is None, "rebuild with debug=False"
    partition_name = (
        nc.partition_id_tensor.name if nc.partition_id_tensor else None
    )

    in_names, out_names, out_avals = [], [], []
    for alloc in nc.m.functions[0].allocations:
        if not isinstance(alloc, mybir.MemoryLocationSet):
            continue
        name = alloc.memorylocations[0].name
        if alloc.kind == "ExternalInput":
            if name != partition_name:
                in_names.append(name)
        elif alloc.kind == "ExternalOutput":
            out_names.append(name)
            out_avals.append(
                jax.core.ShapedArray(
                    tuple(alloc.tensor_shape), mybir.dt.np(alloc.dtype)
                )
            )
    n_params = len(in_names)
    all_in_names = list(in_names) + list(out_names)
    if partition_name is not None:
        all_in_names.append(partition_name)
    donate = tuple(range(n_params, n_params + len(out_names)))

    def _body(*args):
        operands = list(args)
        if partition_name is not None:
            operands.append(bass2jax.partition_id_tensor())
        outs = _bass_exec_p.bind(
            *operands,
            out_avals=tuple(out_avals),
            in_names=tuple(all_in_names),
            out_names=tuple(out_names),
            lowering_input_output_aliases=(),
            sim_require_finite=True,
            sim_require_nnan=True,
            nc=nc,
        )
        return tuple(outs)

    devices = jax.devices()[:NUM_CORES]
    mesh = Mesh(np.asarray(devices), ("core",))
    spec = PartitionSpec("core")
    fn = jax.jit(
        shard_map(
            _body, mesh=mesh,
            in_specs=(spec,) * (n_params + len(out_names)),
            out_specs=(spec,) * len(out_names),
            check_rep=False,
        ),
        donate_argnums=donate, keep_unused=True,
    )

    def run(in_maps):
        concat_in = [
            np.concatenate([np.asarray(m[k]) for m in in_maps], axis=0)
            for k in in_names
        ]
        zeros = [
            np.zeros((NUM_CORES * a.shape[0], *a.shape[1:]), a.dtype)
            for a in out_avals
        ]
        outs = fn(*concat_in, *zeros)
        arr = np.asarray(outs[0]).reshape(NUM_CORES, *out_avals[0].shape)
        return [{out_names[0]: arr[c]} for c in range(NUM_CORES)]

    _RUNNER_CACHE["run"] = run
    return run


def _chunk_in_map(x, w, off: int, size: int, expert: int):
    """Build the per-core input map for one (expert, token-chunk)."""
    xe = np.zeros((CAP, IN_F), np.float32)
    if size > 0:
        xe[:size] = x[off:off + size]
    x8a, xba = prep_x(xe)
    w8a, wba = prep_w(w[expert])
    return {"x8": x8a, "xb": xba, "w8": w8a, "wb": wba}


def kernel(**inputs) -> np.ndarray:
    x = np.asarray(inputs["input_tokens"], dtype=np.float32)       # [T, K]
    w = np.asarray(inputs["weight_stack"], dtype=np.float32)       # [E, O, K]
    m_sizes = np.asarray(inputs["m_sizes"]).astype(np.int64)       # [E]
    m_offsets = np.asarray(inputs["m_offsets"]).astype(np.int64)   # [E]

    T = x.shape[0]
    E, O, K = w.shape
    assert K == IN_F and O == OUT_F and E == NUM_CORES

    # Split each expert's contiguous token group into chunks of <= CAP rows
    # (the deterministic setup gives exactly one CAP-sized chunk per expert).
    chunks = []  # (expert, src_off, size)
    for e in range(E):
        off, size = int(m_offsets[e]), int(m_sizes[e])
        off = max(0, min(off, T))
        size = max(0, min(size, T - off))
        pos = 0
        while pos < size:
            c = min(CAP, size - pos)
            chunks.append((e, off + pos, c))
            pos += c

    out = np.zeros((T, O), dtype=np.float32)
    run = _get_runner()
    for batch_start in range(0, len(chunks), NUM_CORES):
        batch = chunks[batch_start:batch_start + NUM_CORES]
        in_maps = [_chunk_in_map(x, w, off, size, e) for (e, off, size) in batch]
        # SPMD needs a full complement of cores; pad with repeats of map 0.
        while len(in_maps) < NUM_CORES:
            in_maps.append(in_maps[0])
        results = run(in_maps)
        for i, (e, off, size) in enumerate(batch):
            ye = results[i]["y"]  # [CAP, OUT_F] bf16
            out[off:off + size] += ye[:size].astype(np.float32)
    return out


# revision 5
# speedup vs baseline: 1.0100x; 1.0100x over previous
"""Grouped GEMM (MoE expert-parallel) on 8 TRN2 NeuronCores.

Strategy: expert-parallel — core e computes Y_e = X_e @ W_e^T for its
expert's contiguous token group.  Per core: [2048, 1024] @ [1024, 2048].

v4 design — hybrid fp8-DoubleRow / bf16 K-split:
  * Per (m, n) PSUM accumulation group: one DoubleRow fp8e4 matmul
    covers k=0..255 at 2x PE rate (measured 1.9x), then 6 bf16 matmuls
    cover k=256..1023 at 1x.  ~12% fewer PE cycles than all-bf16.
  * Both paths are pre-scaled on host by powers of two (x*32, w*1024)
    so a single fp32 PSUM group accumulates a uniform 32768x scale;
    the PSUM->SBUF copy divides it out (exact).  fp8 covers 1/4 of K,
    so quantization error is halved vs all-fp8: measured rel err
    1.603e-2 on hardware (gate 2e-2; all-fp8 would be 3.2e-2).
  * Host-pre-tiled DMA layouts: every DMA line is per-partition
    contiguous, inputs+outputs are 16-or-8-bit — DMA is fully hidden
    behind the PE (all DMA/copy/order micro-variants measured identical,
    and the full kernel matches the matmul-only floor).
  * Batched-DR issue order (4 DR, then 24 bf16 per m-tile) minimizes
    fp8<->bf16 PE mode switches; scaled PSUM->SBUF copies on the DVE.
"""

import numpy as np
import ml_dtypes

import concourse.mybir as mybir
import concourse.tile as tile
from concourse import bacc

NUM_CORES = 8
IN_F = 1024            # K (contraction)
OUT_F = 2048           # N (out features per expert)
CAP = 2048             # token capacity per core (= expected group size)
P = 128
KT = IN_F // P         # 8 k-subtiles
MT = CAP // P          # 16 m-tiles of tokens
NFREE = 512            # moving-operand free dim (one fp32 PSUM bank)
NT = OUT_F // NFREE    # 4 n-tiles
K8T = 2                # fp8 k-subtiles (one DoubleRow matmul, k=0..255)
KBT = KT - K8T         # bf16 k-subtiles (k=256..1023)
SX, SW = 32.0, 1024.0  # power-of-two pre-scales (exact in fp32/bf16)
INV_SCALE = 1.0 / (SX * SW)

DT8 = mybir.dt.float8e4
DTB = mybir.dt.bfloat16


def _build(repeat: int = 1):
    """Per-core Bass program: y[CAP, OUT_F] (bf16) = X_e @ W_e^T / 32768.

    x8: [MT, P, 2, P]      x8[m,p,j,m'] = e4m3(x[m*128+m', j*128+p]*32)
    xb: [MT, P, 6, P]      xb[m,p,j,m'] = bf16(x[m*128+m', (j+2)*128+p]*32)
    w8: [NT, P, 2, NFREE]  w8[n,p,j,f]  = e4m3(w[n*512+f, j*128+p]*1024)
    wb: [NT, P, 6, NFREE]  wb[n,p,j,f]  = bf16(w[n*512+f, (j+2)*128+p]*1024)
    """
    nc = bacc.Bacc(None, target_bir_lowering=False, debug=False)
    x8 = nc.dram_tensor("x8", [MT, P, K8T, P], DT8, kind="ExternalInput")
    xb = nc.dram_tensor("xb", [MT, P, KBT, P], DTB, kind="ExternalInput")
    w8 = nc.dram_tensor("w8", [NT, P, K8T, NFREE], DT8, kind="ExternalInput")
    wb = nc.dram_tensor("wb", [NT, P, KBT, NFREE], DTB, kind="ExternalInput")
    y = nc.dram_tensor("y", [CAP, OUT_F], DTB, kind="ExternalOutput")
    yr = y.rearrange("(mt p) n -> p mt n", p=P)

    with tile.TileContext(nc) as tc:
        with (
            tc.tile_pool(name="x8_pool", bufs=3) as x8_pool,
            tc.tile_pool(name="xb_pool", bufs=3) as xb_pool,
            tc.tile_pool(name="w_pool", bufs=2 * NT) as w_pool,
            tc.tile_pool(name="out_pool", bufs=3) as out_pool,
            tc.tile_pool(name="psum", bufs=8, space="PSUM") as psum_pool,
        ):
            for _ in range(repeat):
                wt_tiles: dict[int, tuple] = {}
                for m in range(MT):
                    x8_t = x8_pool.tile([P, K8T, P], DT8, tag="x8")
                    nc.sync.dma_start(x8_t[:], x8[m])
                    xb_t = xb_pool.tile([P, KBT, P], DTB, tag="xb")
                    nc.sync.dma_start(xb_t[:], xb[m])
                    y_sb = out_pool.tile([P, OUT_F], DTB, tag="y")
                    psums = []
                    for n in range(NT):
                        if n not in wt_tiles:
                            w8_t = w_pool.tile([P, K8T, NFREE], DT8,
                                               tag="w", name=f"w8_{n}")
                            nc.sync.dma_start(w8_t[:], w8[n])
                            wb_t = w_pool.tile([P, KBT, NFREE], DTB,
                                               tag="w", name=f"wb_{n}")
                            nc.sync.dma_start(wb_t[:], wb[n])
                            wt_tiles[n] = (w8_t, wb_t)
                        psums.append(psum_pool.tile(
                            [P, NFREE], mybir.dt.float32,
                            name=f"psum_{m}_{n}", tag="psum",
                        ))
                    # Batched-DR phase order: all 4 DoubleRow fp8 matmuls
                    # back-to-back (one accumulation-group start per bank),
                    # then the 24 bf16 matmuls n-grouped per bank — only 2
                    # fp8<->bf16 PE mode switches per m-tile (measured
                    # faster than interleaving DR into each group).
                    for n in range(NT):
                        nc.tensor.matmul(
                            psums[n],
                            lhsT=x8_t[:, 0:K8T, :],
                            rhs=wt_tiles[n][0][:, 0:K8T, :],
                            start=True, stop=False,
                            perf_mode=mybir.MatmulPerfMode.DoubleRow,
                        )
                    for n in range(NT):
                        for o in range(KBT):
                            nc.tensor.matmul(
                                psums[n],
                                lhsT=xb_t[:, o, :],
                                rhs=wt_tiles[n][1][:, o, :],
                                start=False, stop=(o == KBT - 1),
                            )
                    for n in range(NT):
                        nc.vector.tensor_scalar_mul(
                            y_sb[:, n * NFREE:(n + 1) * NFREE],
                            psums[n][:], INV_SCALE)
                    nc.sync.dma_start(yr[:, m, :], y_sb[:])
    nc.compile()
    return nc


_NC_CACHE: dict = {}


def _get_nc(repeat: int = 1):
    if repeat not in _NC_CACHE:
        _NC_CACHE[repeat] = _build(repeat)
    return _NC_CACHE[repeat]


def prep_x(xe32: np.ndarray):
    """[CAP, IN_F] fp32 -> (x8 [MT,P,2,P] e4m3, xb [MT,P,6,P] bf16)."""
    t = (xe32 * SX).reshape(MT, P, KT, P).transpose(0, 3, 2, 1)
    x8 = np.clip(t[:, :, :K8T], -240, 240).astype(ml_dtypes.float8_e4m3)
    xb = t[:, :, K8T:].astype(ml_dtypes.bfloat16)
    return np.ascontiguousarray(x8), np.ascontiguousarray(xb)


def prep_w(we32: np.ndarray):
    """[OUT_F, IN_F] fp32 -> (w8 [NT,P,2,NFREE] e4m3, wb [NT,P,6,NFREE] bf16)."""
    t = (we32 * SW).reshape(NT, NFREE, KT, P).transpose(0, 3, 2, 1)
    w8 = np.clip(t[:, :, :K8T], -240, 240).astype(ml_dtypes.float8_e4m3)
    wb = t[:, :, K8T:].astype(ml_dtypes.bfloat16)
    return np.ascontiguousarray(w8), np.ascontiguousarray(wb)


_RUNNER_CACHE: dict = {}


def _get_runner():
    """Jit the 8-core SPMD executable once; reuse across kernel() calls."""
    if "run" in _RUNNER_CACHE:
        return _RUNNER_CACHE["run"]

    import jax
    from jax.sharding import Mesh, PartitionSpec
    from jax.experimental.shard_map import shard_map
    from concourse import bass2jax
    from concourse.bass2jax import _bass_exec_p, install_neuronx_cc_hook

    nc = _get_nc(1)
    install_neuronx_cc_hook()
    assert nc.dbg_addr is None, "rebuild with debug=False"
    partition_name = (
        nc.partition_id_tensor.name if nc.partition_id_tensor else None
    )

    in_names, out_names, out_avals = [], [], []
    for alloc in nc.m.functions[0].allocations:
        if not isinstance(alloc, mybir.MemoryLocationSet):
            continue
        name = alloc.memorylocations[0].name
        if alloc.kind == "ExternalInput":
            if name != partition_name:
                in_names.append(name)
        elif alloc.kind == "ExternalOutput":
            out_names.append(name)
            out_avals.append(
                jax.core.ShapedArray(
                    tuple(alloc.tensor_shape), mybir.dt.np(alloc.dtype)
                )
            )
    n_params = len(in_names)
    all_in_names = list(in_names) + list(out_names)
    if partition_name is not None:
        all_in_names.append(partition_name)
    donate = tuple(range(n_params, n_params + len(out_names)))

    def _body(*args):
        operands = list(args)
        if partition_name is not None:
            operands.append(bass2jax.partition_id_tensor())
        outs = _bass_exec_p.bind(
            *operands,
            out_avals=tuple(out_avals),
            in_names=tuple(all_in_names),
            out_names=tuple(out_names),
            lowering_input_output_aliases=(),
            sim_require_finite=True,
            sim_require_nnan=True,
            nc=nc,
        )
        return tuple(outs)

    devices = jax.devices()[:NUM_CORES]
    mesh = Mesh(np.asarray(devices), ("core",))
    spec = PartitionSpec("core")
    fn = jax.jit(
        shard_map(
            _body, mesh=mesh,
            in_specs=(spec,) * (n_params + len(out_names)),
            out_specs=(spec,) * len(out_names),
            check_rep=False,
        ),
        donate_argnums=donate, keep_unused=True,
    )

    def run(in_maps):
        concat_in = [
            np.concatenate([np.asarray(m[k]) for m in in_maps], axis=0)
            for k in in_names
        ]
        zeros = [
            np.zeros((NUM_CORES * a.shape[0], *a.shape[1:]), a.dtype)
            for a in out_avals
        ]
        outs = fn(*concat_in, *zeros)
        arr = np.asarray(outs[0]).reshape(NUM_CORES, *out_avals[0].shape)
        return [{out_names[0]: arr[c]} for c in range(NUM_CORES)]

    _RUNNER_CACHE["run"] = run
    return run


def _chunk_in_map(x, w, off: int, size: int, expert: int):
    """Build the per-core input map for one (expert, token-chunk)."""
    xe = np.zeros((CAP, IN_F), np.float32)
    if size > 0:
        xe[:size] = x[off:off + size]
    x8a, xba = prep_x(xe)
    w8a, wba = prep_w(w[expert])
    return {"x8": x8a, "xb": xba, "w8": w8a, "wb": wba}


def kernel(**inputs) -> np.ndarray:
    x = np.asarray(inputs["input_tokens"], dtype=np.float32)       # [T, K]
    w = np.asarray(inputs["weight_stack"], dtype=np.float32)       # [E, O, K]
    m_sizes = np.asarray(inputs["m_sizes"]).astype(np.int64)       # [E]
    m_offsets = np.asarray(inputs["m_offsets"]).astype(np.int64)   # [E]

    T = x.shape[0]
    E, O, K = w.shape
    assert K == IN_F and O == OUT_F and E == NUM_CORES

    # Split each expert's contiguous token group into chunks of <= CAP rows
    # (the deterministic setup gives exactly one CAP-sized chunk per expert).
    chunks = []  # (expert, src_off, size)
    for e in range(E):
        off, size = int(m_offsets[e]), int(m_sizes[e])
        off = max(0, min(off, T))
        size = max(0, min(size, T - off))
        pos = 0
        while pos < size:
            c = min(CAP, size - pos)
            chunks.append((e, off + pos, c))
            pos += c

    out = np.zeros((T, O), dtype=np.float32)
    run = _get_runner()
    for batch_start in range(0, len(chunks), NUM_CORES):
        batch = chunks[batch_start:batch_start + NUM_CORES]
        in_maps = [_chunk_in_map(x, w, off, size, e) for (e, off, size) in batch]
        # SPMD needs a full complement of cores; pad with repeats of map 0.
        while len(in_maps) < NUM_CORES:
            in_maps.append(in_maps[0])
        results = run(in_maps)
        for i, (e, off, size) in enumerate(batch):
            ye = results[i]["y"]  # [CAP, OUT_F] bf16
            out[off:off + size] += ye[:size].astype(np.float32)
    return out


# revision 6
# speedup vs baseline: 1.0599x; 1.0495x over previous
"""Grouped GEMM (MoE expert-parallel) on 8 TRN2 NeuronCores.

Strategy: expert-parallel — core e computes Y_e = X_e @ W_e^T for its
expert's contiguous token group.  Per core: [2048, 1024] @ [1024, 2048].

v4 design — hybrid fp8-DoubleRow / bf16 K-split:
  * Per (m, n) PSUM accumulation group: one DoubleRow fp8e4 matmul
    covers k=0..255 at 2x PE rate (measured 1.9x), then 6 bf16 matmuls
    cover k=256..1023 at 1x.  ~12% fewer PE cycles than all-bf16.
  * Both paths are pre-scaled on host by powers of two (x*32, w*1024)
    so a single fp32 PSUM group accumulates a uniform 32768x scale;
    the PSUM->SBUF copy divides it out (exact).  fp8 covers 1/4 of K,
    so quantization error is halved vs all-fp8: measured rel err
    1.603e-2 on hardware (gate 2e-2; all-fp8 would be 3.2e-2).
  * Host-pre-tiled DMA layouts: every DMA line is per-partition
    contiguous, inputs+outputs are 16-or-8-bit — DMA is fully hidden
    behind the PE (all DMA/copy/order micro-variants measured identical,
    and the full kernel matches the matmul-only floor).
  * Batched-DR issue order (4 DR, then 24 bf16 per m-tile) minimizes
    fp8<->bf16 PE mode switches; scaled PSUM->SBUF copies on the DVE.
"""

import numpy as np
import ml_dtypes

import concourse.mybir as mybir
import concourse.tile as tile
from concourse import bacc

NUM_CORES = 8
IN_F = 1024            # K (contraction)
OUT_F = 2048           # N (out features per expert)
CAP = 2048             # token capacity per core (= expected group size)
P = 128
KT = IN_F // P         # 8 k-subtiles
MT = CAP // P          # 16 m-tiles of tokens
NFREE = 512            # moving-operand free dim (one fp32 PSUM bank)
NT = OUT_F // NFREE    # 4 n-tiles
K8T = 2                # fp8 k-subtiles (one DoubleRow matmul, k=0..255)
KBT = KT - K8T         # bf16 k-subtiles (k=256..1023)
SX, SW = 32.0, 1024.0  # power-of-two pre-scales (exact in fp32/bf16)
INV_SCALE = 1.0 / (SX * SW)

DT8 = mybir.dt.float8e4
DTB = mybir.dt.bfloat16


def _build(repeat: int = 1):
    """Per-core Bass program: y[CAP, OUT_F] (bf16) = X_e @ W_e^T / 32768.

    x8: [MT, P, 2, P]      x8[m,p,j,m'] = e4m3(x[m*128+m', j*128+p]*32)
    xb: [MT, P, 6, P]      xb[m,p,j,m'] = bf16(x[m*128+m', (j+2)*128+p]*32)
    w8: [NT, P, 2, NFREE]  w8[n,p,j,f]  = e4m3(w[n*512+f, j*128+p]*1024)
    wb: [NT, P, 6, NFREE]  wb[n,p,j,f]  = bf16(w[n*512+f, (j+2)*128+p]*1024)
    """
    nc = bacc.Bacc(None, target_bir_lowering=False, debug=False)
    x8 = nc.dram_tensor("x8", [MT, P, K8T, P], DT8, kind="ExternalInput")
    xb = nc.dram_tensor("xb", [MT, P, KBT, P], DTB, kind="ExternalInput")
    w8 = nc.dram_tensor("w8", [NT, P, K8T, NFREE], DT8, kind="ExternalInput")
    wb = nc.dram_tensor("wb", [NT, P, KBT, NFREE], DTB, kind="ExternalInput")
    y = nc.dram_tensor("y", [CAP, OUT_F], DTB, kind="ExternalOutput")
    yr = y.rearrange("(mt p) n -> p mt n", p=P)

    with tile.TileContext(nc) as tc:
        with (
            tc.tile_pool(name="x8_pool", bufs=4) as x8_pool,
            tc.tile_pool(name="xb_pool", bufs=4) as xb_pool,
            # two full weight sets: the next repeat's weight DMAs prefetch
            # a whole repeat early instead of stalling the first matmul
            # groups at each repeat boundary (-7.4us/rep measured)
            tc.tile_pool(name="w_pool", bufs=4 * NT) as w_pool,
            tc.tile_pool(name="out_pool", bufs=3) as out_pool,
            tc.tile_pool(name="psum", bufs=8, space="PSUM") as psum_pool,
        ):
            for _ in range(repeat):
                wt_tiles: dict[int, tuple] = {}
                for m in range(MT):
                    x8_t = x8_pool.tile([P, K8T, P], DT8, tag="x8")
                    nc.sync.dma_start(x8_t[:], x8[m])
                    xb_t = xb_pool.tile([P, KBT, P], DTB, tag="xb")
                    nc.sync.dma_start(xb_t[:], xb[m])
                    y_sb = out_pool.tile([P, OUT_F], DTB, tag="y")
                    psums = []
                    for n in range(NT):
                        if n not in wt_tiles:
                            w8_t = w_pool.tile([P, K8T, NFREE], DT8,
                                               tag="w", name=f"w8_{n}")
                            nc.sync.dma_start(w8_t[:], w8[n])
                            wb_t = w_pool.tile([P, KBT, NFREE], DTB,
                                               tag="w", name=f"wb_{n}")
                            nc.sync.dma_start(wb_t[:], wb[n])
                            wt_tiles[n] = (w8_t, wb_t)
                        psums.append(psum_pool.tile(
                            [P, NFREE], mybir.dt.float32,
                            name=f"psum_{m}_{n}", tag="psum",
                        ))
                    # Batched-DR phase order: all 4 DoubleRow fp8 matmuls
                    # back-to-back (one accumulation-group start per bank),
                    # then the 24 bf16 matmuls n-grouped per bank — only 2
                    # fp8<->bf16 PE mode switches per m-tile (measured
                    # faster than interleaving DR into each group).
                    for n in range(NT):
                        nc.tensor.matmul(
                            psums[n],
                            lhsT=x8_t[:, 0:K8T, :],
                            rhs=wt_tiles[n][0][:, 0:K8T, :],
                            start=True, stop=False,
                            perf_mode=mybir.MatmulPerfMode.DoubleRow,
                        )
                    for n in range(NT):
                        for o in range(KBT):
                            nc.tensor.matmul(
                                psums[n],
                                lhsT=xb_t[:, o, :],
                                rhs=wt_tiles[n][1][:, o, :],
                                start=False, stop=(o == KBT - 1),
                            )
                    for n in range(NT):
                        nc.vector.tensor_scalar_mul(
                            y_sb[:, n * NFREE:(n + 1) * NFREE],
                            psums[n][:], INV_SCALE)
                    nc.sync.dma_start(yr[:, m, :], y_sb[:])
    nc.compile()
    return nc


_NC_CACHE: dict = {}


def _get_nc(repeat: int = 1):
    if repeat not in _NC_CACHE:
        _NC_CACHE[repeat] = _build(repeat)
    return _NC_CACHE[repeat]


def prep_x(xe32: np.ndarray):
    """[CAP, IN_F] fp32 -> (x8 [MT,P,2,P] e4m3, xb [MT,P,6,P] bf16)."""
    t = (xe32 * SX).reshape(MT, P, KT, P).transpose(0, 3, 2, 1)
    x8 = np.clip(t[:, :, :K8T], -240, 240).astype(ml_dtypes.float8_e4m3)
    xb = t[:, :, K8T:].astype(ml_dtypes.bfloat16)
    return np.ascontiguousarray(x8), np.ascontiguousarray(xb)


def prep_w(we32: np.ndarray):
    """[OUT_F, IN_F] fp32 -> (w8 [NT,P,2,NFREE] e4m3, wb [NT,P,6,NFREE] bf16)."""
    t = (we32 * SW).reshape(NT, NFREE, KT, P).transpose(0, 3, 2, 1)
    w8 = np.clip(t[:, :, :K8T], -240, 240).astype(ml_dtypes.float8_e4m3)
    wb = t[:, :, K8T:].astype(ml_dtypes.bfloat16)
    return np.ascontiguousarray(w8), np.ascontiguousarray(wb)


_RUNNER_CACHE: dict = {}


def _get_runner():
    """Jit the 8-core SPMD executable once; reuse across kernel() calls."""
    if "run" in _RUNNER_CACHE:
        return _RUNNER_CACHE["run"]

    import jax
    from jax.sharding import Mesh, PartitionSpec
    from jax.experimental.shard_map import shard_map
    from concourse import bass2jax
    from concourse.bass2jax import _bass_exec_p, install_neuronx_cc_hook

    nc = _get_nc(1)
    install_neuronx_cc_hook()
    assert nc.dbg_addr is None, "rebuild with debug=False"
    partition_name = (
        nc.partition_id_tensor.name if nc.partition_id_tensor else None
    )

    in_names, out_names, out_avals = [], [], []
    for alloc in nc.m.functions[0].allocations:
        if not isinstance(alloc, mybir.MemoryLocationSet):
            continue
        name = alloc.memorylocations[0].name
        if alloc.kind == "ExternalInput":
            if name != partition_name:
                in_names.append(name)
        elif alloc.kind == "ExternalOutput":
            out_names.append(name)
            out_avals.append(
                jax.core.ShapedArray(
                    tuple(alloc.tensor_shape), mybir.dt.np(alloc.dtype)
                )
            )
    n_params = len(in_names)
    all_in_names = list(in_names) + list(out_names)
    if partition_name is not None:
        all_in_names.append(partition_name)
    donate = tuple(range(n_params, n_params + len(out_names)))

    def _body(*args):
        operands = list(args)
        if partition_name is not None:
            operands.append(bass2jax.partition_id_tensor())
        outs = _bass_exec_p.bind(
            *operands,
            out_avals=tuple(out_avals),
            in_names=tuple(all_in_names),
            out_names=tuple(out_names),
            lowering_input_output_aliases=(),
            sim_require_finite=True,
            sim_require_nnan=True,
            nc=nc,
        )
        return tuple(outs)

    devices = jax.devices()[:NUM_CORES]
    mesh = Mesh(np.asarray(devices), ("core",))
    spec = PartitionSpec("core")
    fn = jax.jit(
        shard_map(
            _body, mesh=mesh,
            in_specs=(spec,) * (n_params + len(out_names)),
            out_specs=(spec,) * len(out_names),
            check_rep=False,
        ),
        donate_argnums=donate, keep_unused=True,
    )

    def run(in_maps):
        concat_in = [
            np.concatenate([np.asarray(m[k]) for m in in_maps], axis=0)
            for k in in_names
        ]
        zeros = [
            np.zeros((NUM_CORES * a.shape[0], *a.shape[1:]), a.dtype)
            for a in out_avals
        ]
        outs = fn(*concat_in, *zeros)
        arr = np.asarray(outs[0]).reshape(NUM_CORES, *out_avals[0].shape)
        return [{out_names[0]: arr[c]} for c in range(NUM_CORES)]

    _RUNNER_CACHE["run"] = run
    return run


def _chunk_in_map(x, w, off: int, size: int, expert: int):
    """Build the per-core input map for one (expert, token-chunk)."""
    xe = np.zeros((CAP, IN_F), np.float32)
    if size > 0:
        xe[:size] = x[off:off + size]
    x8a, xba = prep_x(xe)
    w8a, wba = prep_w(w[expert])
    return {"x8": x8a, "xb": xba, "w8": w8a, "wb": wba}


def kernel(**inputs) -> np.ndarray:
    x = np.asarray(inputs["input_tokens"], dtype=np.float32)       # [T, K]
    w = np.asarray(inputs["weight_stack"], dtype=np.float32)       # [E, O, K]
    m_sizes = np.asarray(inputs["m_sizes"]).astype(np.int64)       # [E]
    m_offsets = np.asarray(inputs["m_offsets"]).astype(np.int64)   # [E]

    T = x.shape[0]
    E, O, K = w.shape
    assert K == IN_F and O == OUT_F and E == NUM_CORES

    # Split each expert's contiguous token group into chunks of <= CAP rows
    # (the deterministic setup gives exactly one CAP-sized chunk per expert).
    chunks = []  # (expert, src_off, size)
    for e in range(E):
        off, size = int(m_offsets[e]), int(m_sizes[e])
        off = max(0, min(off, T))
        size = max(0, min(size, T - off))
        pos = 0
        while pos < size:
            c = min(CAP, size - pos)
            chunks.append((e, off + pos, c))
            pos += c

    out = np.zeros((T, O), dtype=np.float32)
    run = _get_runner()
    for batch_start in range(0, len(chunks), NUM_CORES):
        batch = chunks[batch_start:batch_start + NUM_CORES]
        in_maps = [_chunk_in_map(x, w, off, size, e) for (e, off, size) in batch]
        # SPMD needs a full complement of cores; pad with repeats of map 0.
        while len(in_maps) < NUM_CORES:
            in_maps.append(in_maps[0])
        results = run(in_maps)
        for i, (e, off, size) in enumerate(batch):
            ye = results[i]["y"]  # [CAP, OUT_F] bf16
            out[off:off + size] += ye[:size].astype(np.float32)
    return out
